# revision 1
# baseline (speedup 1.0000x reference)
"""Bass/Tile kernel for EpisodeMultiheadAttentionBlock on TRN2.

Per-core: 2 batch elements (data-parallel over B=16 across 8 cores).

Layout: activations feature-major [feature parts, token free]; V token-major.
All matmuls float32r. Softmax without max-subtraction; multiplicative 0/1
mask after exp; 1/rowsum via ones-matmul (which also broadcasts it across
all partitions); pT normalized in-place so attn@V and attn_w both consume
normalized attention weights.
"""

import os

import numpy as np

import concourse.bass as bass
import concourse.mybir as mybir
import concourse.tile as tile
from concourse import bacc
from concourse.tile_rust import add_dep_helper

F32 = mybir.dt.float32
F32R = mybir.dt.float32r
BF16 = mybir.dt.bfloat16
AF = mybir.ActivationFunctionType

B_PER_CORE = 2
L = 1024
E = 1024
H = 16
QL = 256
NCH = 8

SINGLES = ["wq", "wk", "wv", "wo"]
GRUCATS = ["wrC", "wzC", "wgC"]


def build_kernel(num_devices=8):
    nc = bacc.Bacc("TRN2", target_bir_lowering=False, debug=False,
                   num_devices=num_devices)

    kinT_d = nc.dram_tensor("kinT", [B_PER_CORE, E, L], BF16, kind="ExternalInput")
    keyT_d = nc.dram_tensor("keyT", [B_PER_CORE, E, L], BF16, kind="ExternalInput")
    maskT_d = nc.dram_tensor("maskT", [B_PER_CORE, L, QL], mybir.dt.bfloat16, kind="ExternalInput")
    ones_d = nc.dram_tensor("ones", [128, 128], BF16, kind="ExternalInput")
    onesf_d = nc.dram_tensor("onesf", [1, 128], F32R, kind="ExternalInput")
    zeros_d = nc.dram_tensor("zeros", [128, QL], BF16, kind="ExternalInput")
    w_d = {n: nc.dram_tensor(n + "T", [E, E], BF16, kind="ExternalInput")
           for n in SINGLES}
    for n in GRUCATS:
        w_d[n] = nc.dram_tensor(n, [2 * E, E], BF16, kind="ExternalInput")
    b_d = {n: nc.dram_tensor(n, [E], F32, kind="ExternalInput")
           for n in ["bq", "bk", "bo", "brz", "bzz", "bgg"]}
    bv_d = nc.dram_tensor("bv", [1, E], F32, kind="ExternalInput")

    kts_d = nc.dram_tensor("kts", [B_PER_CORE, NCH, 128, L], BF16)
    outT_d = nc.dram_tensor("outT", [B_PER_CORE, E, QL], F32, kind="ExternalOutput")
    attnwT_d = nc.dram_tensor("attnwT", [B_PER_CORE, L, QL], F32,
                              kind="ExternalOutput")

    with tile.TileContext(nc) as tc, nc.allow_low_precision(
            reason="float32r tiles feed f32r matmuls; PSUM accumulation is f32"):
        _body(nc, tc, kinT_d, keyT_d, maskT_d, ones_d, onesf_d, zeros_d,
              w_d, b_d, bv_d, kts_d, outT_d, attnwT_d)
    nc.compile()
    return nc


def _hbcast(ap, n):
    """Broadcast a [P, Q] AP to [P, n, Q] with a step-0 middle dim."""
    lst = [list(p) for p in ap.ap]
    return bass.AP(tensor=ap.tensor, offset=ap.offset,
                   ap=[lst[0], [0, n]] + lst[1:])


def _body(nc, tc, kinT_d, keyT_d, maskT_d, ones_d, onesf_d, zeros_d,
          w_d, b_d, bv_d, kts_d, outT_d, attnwT_d):
    from contextlib import ExitStack
    ctx = ExitStack()
    with ctx:
        consts = ctx.enter_context(tc.tile_pool(name="consts", bufs=1))
        wpool = ctx.enter_context(tc.tile_pool(name="wpool", bufs=4))
        big = ctx.enter_context(tc.tile_pool(name="big", bufs=1))
        med = ctx.enter_context(tc.tile_pool(name="med", bufs=1))
        small = ctx.enter_context(tc.tile_pool(name="small", bufs=2))
        ps_proj = ctx.enter_context(
            tc.tile_pool(name="ps_proj", bufs=5, space="PSUM"))
        ps_half = ctx.enter_context(
            tc.tile_pool(name="ps_half", bufs=3, space="PSUM"))

        ones_t = consts.tile([128, 128], BF16, tag="ones")
        nc.sync.dma_start(ones_t[:], ones_d[:])
        onesf_t = consts.tile([1, 128], F32R, tag="onesf")
        nc.sync.dma_start(onesf_t[:], onesf_d[:])
        zeros_t = consts.tile([128, QL], BF16, tag="zeros")
        nc.sync.dma_start(zeros_t[:], zeros_d[:])
        bias_sb = {}
        for n, d in b_d.items():
            t = consts.tile([128, NCH], F32, tag="b_" + n)
            nc.sync.dma_start(t[:], d.rearrange("(c p) -> p c", p=128))
            bias_sb[n] = t
        bvb = consts.tile([128, E], F32, tag="bvb")
        bv_ap = bv_d[:, :]
        nc.gpsimd.dma_start(
            out=bvb[:],
            in_=bass.AP(tensor=bv_ap.tensor, offset=bv_ap.offset,
                        ap=[[0, 128], [1, E]]))
        def _bc2(ap, n1, n2):
            """Broadcast a [P, Q] AP to [P, n1, n2, Q] with step-0 dims."""
            lst = [list(p) for p in ap.ap]
            return bass.AP(tensor=ap.tensor, offset=ap.offset,
                           ap=[lst[0], [0, n1], [0, n2]] + lst[1:])

        def load_wq(name, qo):
            """Quarter qo (out-cols qo*256..+256) of a single [E,E] weight."""
            t = wpool.tile([128, NCH, 256], BF16, tag="w", name=f"{name}_{qo}")
            nc.sync.dma_start(
                t[:],
                w_d[name][:, qo * 256:(qo + 1) * 256]
                .rearrange("(c p) o -> p c o", p=128))
            return t

        def load_w8(name, mo):
            """Eighth mo (out-cols mo*128..+128) of a [2E,E] GRU cat weight."""
            t = wpool.tile([128, 2 * NCH, 128], BF16, tag="w",
                           name=f"{name}_{mo}")
            nc.sync.dma_start(
                t[:],
                w_d[name][:, mo * 128:(mo + 1) * 128]
                .rearrange("(c p) o -> p c o", p=128))
            return t

        NB = B_PER_CORE

        # ---- batch-concat inputs: qin (kin last QL), oriq (key last QL) ----
        qin = med.tile([128, NB, NCH, QL], BF16, tag="qin", bufs=1)
        oriq = med.tile([128, NB, NCH, QL], BF16, tag="oriq", bufs=1)
        for b in range(NB):
            nc.sync.dma_start(
                qin[:, b], kinT_d[b][:, L - QL:]
                .rearrange("(c p) t -> p c t", p=128))
            nc.sync.dma_start(
                oriq[:, b], keyT_d[b][:, L - QL:]
                .rearrange("(c p) t -> p c t", p=128))

        # ---- q projection (both batches, block-diag head-pair layout) ----
        # qTz[p, b, g, hi, t]: quadrant (0:64, hi=0) = even head, (64:128,
        # hi=1) = odd head of pair g; opposite quadrants are zero so the
        # scores matmul can contract all 128 partitions.
        qTz = med.tile([128, NB, NCH, 2, QL], BF16, tag="qTz", bufs=1)
        nc.vector.memset(qTz[64:128, :, :, 0, :], 0.0)
        nc.vector.memset(qTz[0:64, :, :, 1, :], 0.0)
        wq_q = [load_wq("wq", qo) for qo in range(4)]
        for mo in range(NCH):
            p = ps_proj.tile([128, NB, QL], F32, tag="proj")
            wt = wq_q[mo // 2]
            for ci in range(NCH):
                nc.tensor.matmul(
                    p[:].rearrange("p a b -> p (a b)"),
                    wt[:, ci, (mo % 2) * 128:(mo % 2) * 128 + 128],
                    qin[:, :, ci, :],
                    start=(ci == 0), stop=(ci == NCH - 1))
            nc.scalar.activation(qTz[0:64, :, mo, 0, :], p[0:64], AF.Identity,
                                 bias=bias_sb["bq"][0:64, mo:mo + 1])
            nc.scalar.activation(qTz[64:128, :, mo, 1, :], p[64:128],
                                 AF.Identity,
                                 bias=bias_sb["bq"][64:128, mo:mo + 1])

        aoutT = med.tile([128, NB, NCH, QL], BF16, tag="aoutT", bufs=1)

        # ================= attention (per batch) =================
        for b in range(NB):
            kin = big.tile([128, NCH, L], BF16, tag="kin", bufs=2,
                           name=f"kin{b}")
            nc.sync.dma_start(kin[:],
                              kinT_d[b].rearrange("(c p) t -> p c t", p=128))
            maskt = big.tile([128, NCH, QL], BF16, tag="mask", bufs=2,
                             name=f"mask{b}")
            nc.sync.dma_start(maskt[:],
                              maskT_d[b].rearrange("(c p) t -> p c t", p=128))

            # ---- k projection (streamed to DRAM scratch) ----
            wk_q = [load_wq("wk", qo) for qo in range(4)]
            for mo in range(NCH):
                wt = wk_q[mo // 2]
                for n in range(2):
                    p = ps_proj.tile([128, 512], F32, tag="proj")
                    for ci in range(NCH):
                        nc.tensor.matmul(
                            p[:], wt[:, ci, (mo % 2) * 128:(mo % 2) * 128 + 128],
                            kin[:, ci, n * 512:(n + 1) * 512],
                            start=(ci == 0), stop=(ci == NCH - 1))
                    kt = small.tile([128, 512], BF16, tag="ktmp", bufs=2)
                    nc.scalar.activation(kt[:], p[:], AF.Identity,
                                         bias=bias_sb["bk"][:, mo:mo + 1])
                    nc.sync.dma_start(kts_d[b, mo, :, n * 512:(n + 1) * 512],
                                      kt[:])

            # ---- v projection (token-major, 65th column = ones for the
            # softmax row-sum); key streamed ----
            wv_q = [load_wq("wv", qo) for qo in range(4)]
            vkm = big.tile([128, NCH, 2 * NCH, 65], BF16, tag="vkm", bufs=1,
                           name=f"vkm{b}")
            nc.vector.memset(vkm[:, :, :, 64:65], 1.0)
            for kc in range(NCH):
                keyc = small.tile([128, NCH, 128], BF16, tag="keyc", bufs=1)
                nc.sync.dma_start(
                    keyc[:],
                    keyT_d[b][:, kc * 128:(kc + 1) * 128]
                    .rearrange("(c p) t -> p c t", p=128))
                for q4 in range(4):
                    p = ps_half.tile([128, 256], F32, tag="half")
                    for ci in range(NCH):
                        nc.tensor.matmul(
                            p[:], keyc[:, ci, :], wv_q[q4][:, ci, :],
                            start=(ci == 0), stop=(ci == NCH - 1))
                    nc.vector.tensor_add(
                        vkm[:, kc, 4 * q4:4 * q4 + 4, 0:64],
                        p[:].rearrange("p (h d) -> p h d", h=4),
                        bvb[:, q4 * 256:(q4 + 1) * 256]
                        .rearrange("p (h d) -> p h d", h=4))

            # ---- attention, per 2-head group g ----
            awT = med.tile([128, NCH, QL], F32, tag="awT", bufs=1,
                           name=f"awT{b}")
            # pins guard the pT slot two generations back (pT bufs=2)
            pin_pe = [None, None]   # last attn@V matmul per parity
            pin_dve = [None, None]  # last awtmp mul per parity
            for g in range(NCH):
                ktd = med.tile([128, L], BF16, tag="ktd", bufs=2,
                               name=f"ktd{b}_{g}")
                nc.sync.dma_start(ktd[:], kts_d[b, g])
                pT = med.tile([128, NCH, 2, QL], BF16, tag="pT", bufs=2,
                              name=f"pT{b}_{g}")
                for kc in range(NCH):
                    ps = ps_proj.tile([128, 2, QL], F32, tag="proj")
                    nc.tensor.matmul(
                        ps[:].rearrange("p a b -> p (a b)"),
                        ktd[:, kc * 128:(kc + 1) * 128],
                        qTz[:, b, g].rearrange("p a b -> p (a b)"),
                        start=True, stop=True)
                    nc.scalar.activation(pT[:, kc], ps[:], AF.Exp, scale=0.125)
                    mi = nc.vector.tensor_mul(pT[:, kc], pT[:, kc],
                                              _hbcast(maskt[:, kc, :], 2))
                    if kc == 0 and pin_dve[g % 2] is not None:
                        add_dep_helper(mi.ins, pin_dve[g % 2].ins, sync=False,
                                       reason="order pT DVE readers across groups")

                # attn @ V per head (M=65; row 64 = softmax denominator)
                pav = [ps_half.tile([65, QL], F32, tag="half",
                                    name=f"pav{hi}") for hi in range(2)]
                for kc in range(NCH):
                    for hi in range(2):
                        av = nc.tensor.matmul(
                            pav[hi][:, :],
                            vkm[:, kc, 2 * g + hi, :],
                            pT[:, kc, hi, :],
                            start=(kc == 0), stop=(kc == NCH - 1))
                        if kc == 0 and hi == 0 and pin_pe[g % 2] is not None:
                            add_dep_helper(av.ins, pin_pe[g % 2].ins, sync=False,
                                           reason="order pT PE readers across groups")
                pin_pe[g % 2] = av

                # row-sums sit on partition 64 (DVE lanes can't cross
                # partitions): copy to SBUF at partition 64, reciprocal in
                # place, broadcast via f32r ones-matmul whose lhsT also
                # lives at partition 64
                rs = small.tile([65, 2, QL], F32, tag="rs", bufs=1)
                nc.vector.tensor_copy(rs[64:65, 0, :], pav[0][64:65, :])
                nc.vector.tensor_copy(rs[64:65, 1, :], pav[1][64:65, :])
                rs0 = small.tile([1, 2, QL], F32, tag="rs0", bufs=1)
                nc.sync.dma_start(rs0[:], rs[64:65])
                nc.vector.reciprocal_approx_fast(rs0[:], rs0[:])
                r1 = small.tile([1, 2, QL], F32R, tag="r1", bufs=1)
                nc.vector.tensor_copy(r1[:], rs0[:])
                rbp = ps_proj.tile([128, 2, QL], F32, tag="proj")
                nc.tensor.matmul(rbp[:].rearrange("p a b -> p (a b)"),
                                 onesf_t[:],
                                 r1[:].rearrange("p a b -> p (a b)"),
                                 start=True, stop=True)
                recipb = small.tile([64, 2, QL], F32, tag="recipb", bufs=1)
                nc.vector.tensor_copy(recipb[:], rbp[0:64])
                rbpb = small.tile([128, 2, QL], BF16, tag="rbpb", bufs=1)
                nc.vector.tensor_copy(rbpb[:], rbp[:])

                # normalize attn@V during eviction; odd head partition-shifted
                nc.vector.tensor_mul(aoutT[0:64, b, g, :], pav[0][0:64, :],
                                     recipb[:, 0, :])
                sh = small.tile([64, QL], BF16, tag="btmp")
                nc.vector.tensor_mul(sh[:, :], pav[1][0:64, :],
                                     recipb[:, 1, :])
                nc.sync.dma_start(aoutT[64:128, b, g, :], sh[:, :])

                # attn_w: one fused normalize-mul; head-sum adds on GpSimd
                tmp = med.tile([128, NCH, 2, QL], BF16, tag="awtmp", bufs=2,
                               name=f"awtmp{g % 2}")
                lm = nc.vector.tensor_mul(tmp[:], pT[:], _hbcast(rbpb[:], NCH))
                pin_dve[g % 2] = lm
                if g == 0:
                    nc.gpsimd.tensor_add(awT[:], tmp[:, :, 0, :],
                                         tmp[:, :, 1, :])
                else:
                    nc.gpsimd.tensor_add(awT[:], awT[:], tmp[:, :, 0, :])
                    nc.gpsimd.tensor_add(awT[:], awT[:], tmp[:, :, 1, :])

            nc.scalar.activation(awT[:], awT[:], AF.Copy, scale=1.0 / H)
            nc.sync.dma_start(
                attnwT_d[b].rearrange("(c p) t -> p c t", p=128), awT[:])

        # ================= out proj + GRU (batch-concat) =================
        wo_q = [load_wq("wo", qo) for qo in range(4)]
        outT = med.tile([128, NB, NCH, QL], BF16, tag="outT", bufs=1)
        for mo in range(NCH):
            p = ps_proj.tile([128, NB, QL], F32, tag="proj")
            wt = wo_q[mo // 2]
            for ci in range(NCH):
                nc.tensor.matmul(
                    p[:].rearrange("p a b -> p (a b)"),
                    wt[:, ci, (mo % 2) * 128:(mo % 2) * 128 + 128],
                    aoutT[:, :, ci, :], start=(ci == 0), stop=(ci == NCH - 1))
            t = small.tile([128, NB, QL], F32, tag="btmp2")
            nc.scalar.activation(t[:], p[:], AF.Relu,
                                 bias=bias_sb["bo"][:, mo:mo + 1])
            nc.vector.tensor_add(outT[:, :, mo, :], t[:], oriq[:, :, mo, :])

        # r-gate: rq = relu([oriq;outT] @ wrC + brz) * oriq
        rqT = med.tile([128, NB, NCH, QL], BF16, tag="rqT", bufs=1)
        hT = med.tile([128, NB, NCH, QL], BF16, tag="hT", bufs=1)

        for stage, (wname, bias, func) in enumerate(
                [("wrC", "brz", AF.Relu), ("wgC", "bgg", AF.Tanh),
                 ("wzC", "bzz", AF.Relu)]):
            xside = rqT if stage == 1 else oriq
            for mo in range(NCH):
                w8 = load_w8(wname, mo)
                p = ps_proj.tile([128, NB, QL], F32, tag="proj")
                for ci in range(2 * NCH):
                    rhs = (xside[:, :, ci, :] if ci < NCH
                           else outT[:, :, ci - NCH, :])
                    nc.tensor.matmul(
                        p[:].rearrange("p a b -> p (a b)"), w8[:, ci, :], rhs,
                        start=(ci == 0), stop=(ci == 2 * NCH - 1))
                if stage == 0:   # r -> rq
                    t = small.tile([128, NB, QL], F32, tag="btmp2")
                    nc.scalar.activation(t[:], p[:], func,
                                         bias=bias_sb[bias][:, mo:mo + 1])
                    nc.vector.tensor_mul(rqT[:, :, mo, :], t[:],
                                         oriq[:, :, mo, :])
                elif stage == 1:  # h
                    nc.scalar.activation(hT[:, :, mo, :], p[:], func,
                                         bias=bias_sb[bias][:, mo:mo + 1])
                else:            # z + final blend + store
                    zt = small.tile([128, NB, QL], F32, tag="btmp2")
                    nc.scalar.activation(zt[:], p[:], func,
                                         bias=bias_sb[bias][:, mo:mo + 1])
                    d = small.tile([128, NB, QL], F32, tag="btmp2")
                    nc.gpsimd.tensor_sub(d[:], hT[:, :, mo, :],
                                         oriq[:, :, mo, :])
                    nc.gpsimd.tensor_mul(d[:], d[:], zt[:])
                    fin = small.tile([128, NB, QL], F32, tag="btmp2")
                    nc.gpsimd.tensor_add(fin[:], d[:], oriq[:, :, mo, :])
                    for b in range(NB):
                        nc.sync.dma_start(
                            outT_d[b][mo * 128:(mo + 1) * 128, :], fin[:, b, :])


def prep_inputs_core(core, key, pe, key_index, key_padding_mask,
                     in_proj_w, in_proj_b, out_w, out_b, gw, gb):
    b0 = core * B_PER_CORE
    sl = slice(b0, b0 + B_PER_CORE)
    keyc = np.asarray(key[sl], np.float32)
    kin = keyc + np.asarray(pe[sl], np.float32)
    kinT = np.ascontiguousarray(kin.transpose(0, 2, 1))
    keyT = np.ascontiguousarray(keyc.transpose(0, 2, 1))

    ki = np.asarray(key_index[sl])
    pad = np.asarray(key_padding_mask[sl])
    qi = ki[:, L - QL:]
    ri = ki[:, :L - QL]
    import ml_dtypes
    bf16 = ml_dtypes.bfloat16
    allowed = np.zeros((B_PER_CORE, L, QL), np.float32)
    allowed[:, :L - QL, :] = ((ri[:, :, None] < qi[:, None, :])
                              & ~pad[:, :L - QL, None])
    allowed[:, L - QL:, :] = np.eye(QL, dtype=np.float32)[None]

    w32 = lambda x: np.asarray(x, np.float32)
    wbf = lambda x: np.ascontiguousarray(x).astype(bf16)
    im = {
        "kinT": wbf(kinT), "keyT": wbf(keyT),
        "maskT": allowed.astype(bf16),
        "ones": np.ones((128, 128), bf16),
        "onesf": np.ones((1, 128), np.float32),
        "zeros": np.zeros((128, QL), bf16),
        "bv": w32(in_proj_b[2 * E:]).reshape(1, E),
        "bq": w32(in_proj_b[:E]),
        "bk": w32(in_proj_b[E:2 * E]),
        "bo": w32(out_b),
        "brz": w32(gb["bxr"] + gb["byr"]),
        "bzz": w32(gb["bxz"] + gb["byz"]),
        "bgg": w32(gb["bxg"] + gb["byg"]),
        "wqT": wbf(w32(in_proj_w[:E]).T),
        "wkT": wbf(w32(in_proj_w[E:2 * E]).T),
        "wvT": wbf(w32(in_proj_w[2 * E:]).T),
        "woT": wbf(w32(out_w).T),
        "wrC": wbf(np.concatenate([w32(gw["wxr"]).T, w32(gw["wyr"]).T], 0)),
        "wzC": wbf(np.concatenate([w32(gw["wxz"]).T, w32(gw["wyz"]).T], 0)),
        "wgC": wbf(np.concatenate([w32(gw["wxg"]).T, w32(gw["wyg"]).T], 0)),
    }
    return im


def postprocess(results):
    outs, aws = [], []
    for r in results:
        outs.append(r["outT"].transpose(0, 2, 1))
        aws.append(r["attnwT"].transpose(0, 2, 1))
    return (np.concatenate(outs, 0), np.concatenate(aws, 0))


_NC_CACHE = {}


def kernel(key, pe, key_index, key_padding_mask, query_length,
           in_proj_w, in_proj_b, out_w, out_b,
           wxr, bxr, wyr, byr, wxz, bxz, wyz, byz, wxg, bxg, wyg, byg):
    """Full-input entry point: shard B=16 across 8 NeuronCores, run, gather."""
    from concourse.bass_utils import run_bass_kernel_spmd

    key = np.asarray(key)
    assert int(query_length) == QL and key.shape == (16, L, E)
    if "nc" not in _NC_CACHE:
        _NC_CACHE["nc"] = build_kernel(num_devices=8)
    nc = _NC_CACHE["nc"]

    gw = {"wxr": wxr, "wyr": wyr, "wxz": wxz, "wyz": wyz,
          "wxg": wxg, "wyg": wyg}
    gb = {"bxr": bxr, "byr": byr, "bxz": bxz, "byz": byz,
          "bxg": bxg, "byg": byg}
    in_maps = [prep_inputs_core(c, key, pe, key_index, key_padding_mask,
                                in_proj_w, in_proj_b, out_w, out_b, gw, gb)
               for c in range(8)]
    res = run_bass_kernel_spmd(nc, in_maps, core_ids=list(range(8)))
    out, attn_w = postprocess(res.results)
    return out.astype(np.float32), attn_w.astype(np.float32)



# revision 16
# speedup vs baseline: 1.0123x; 1.0123x over previous
"""Bass/Tile kernel for EpisodeMultiheadAttentionBlock on TRN2.

Per-core: 2 batch elements (data-parallel over B=16 across 8 cores).

Layout: activations feature-major [feature parts, token free]; V token-major.
All matmuls float32r. Softmax without max-subtraction; multiplicative 0/1
mask after exp; 1/rowsum via ones-matmul (which also broadcasts it across
all partitions); pT normalized in-place so attn@V and attn_w both consume
normalized attention weights.
"""

import os

import numpy as np

import concourse.bass as bass
import concourse.mybir as mybir
import concourse.tile as tile
from concourse import bacc
from concourse.tile_rust import add_dep_helper

F32 = mybir.dt.float32
F32R = mybir.dt.float32r
BF16 = mybir.dt.bfloat16
AF = mybir.ActivationFunctionType

B_PER_CORE = 2
L = 1024
E = 1024
H = 16
QL = 256
NCH = 8

SINGLES = ["wq", "wk", "wv", "wo"]
GRUCATS = ["wrC", "wzC", "wgC"]


def build_kernel(num_devices=8):
    nc = bacc.Bacc("TRN2", target_bir_lowering=False, debug=False,
                   num_devices=num_devices)

    kinT_d = nc.dram_tensor("kinT", [B_PER_CORE, E, L], BF16, kind="ExternalInput")
    keyT_d = nc.dram_tensor("keyT", [B_PER_CORE, E, L], BF16, kind="ExternalInput")
    maskT_d = nc.dram_tensor("maskT", [B_PER_CORE, L, QL], mybir.dt.bfloat16, kind="ExternalInput")
    ones_d = nc.dram_tensor("ones", [128, 128], BF16, kind="ExternalInput")
    onesf_d = nc.dram_tensor("onesf", [1, 128], F32R, kind="ExternalInput")
    w_d = {n: nc.dram_tensor(n + "T", [E, E], BF16, kind="ExternalInput")
           for n in SINGLES}
    for n in GRUCATS:
        w_d[n] = nc.dram_tensor(n, [2 * E, E], BF16, kind="ExternalInput")
    b_d = {n: nc.dram_tensor(n, [E], F32, kind="ExternalInput")
           for n in ["bq", "bk", "bo", "brz", "bzz", "bgg"]}
    bv_d = nc.dram_tensor("bv", [1, E], F32, kind="ExternalInput")

    kts_d = nc.dram_tensor("kts", [B_PER_CORE, NCH, 128, L], BF16)
    outT_d = nc.dram_tensor("outT", [B_PER_CORE, E, QL], F32, kind="ExternalOutput")
    attnwT_d = nc.dram_tensor("attnwT", [B_PER_CORE, L, QL], F32,
                              kind="ExternalOutput")

    with tile.TileContext(nc) as tc, nc.allow_low_precision(
            reason="float32r tiles feed f32r matmuls; PSUM accumulation is f32"):
        _body(nc, tc, kinT_d, keyT_d, maskT_d, ones_d, onesf_d,
              w_d, b_d, bv_d, kts_d, outT_d, attnwT_d)
    nc.compile()
    return nc


def _hbcast(ap, n):
    """Broadcast a [P, Q] AP to [P, n, Q] with a step-0 middle dim."""
    lst = [list(p) for p in ap.ap]
    return bass.AP(tensor=ap.tensor, offset=ap.offset,
                   ap=[lst[0], [0, n]] + lst[1:])


def _bc_hi2(ap):
    """Broadcast a [P, C, Q] AP to [P, C, 2, Q] with a step-0 hi dim."""
    lst = [list(p) for p in ap.ap]
    return bass.AP(tensor=ap.tensor, offset=ap.offset,
                   ap=[lst[0], lst[1], [0, 2]] + lst[2:])


def _body(nc, tc, kinT_d, keyT_d, maskT_d, ones_d, onesf_d,
          w_d, b_d, bv_d, kts_d, outT_d, attnwT_d):
    from contextlib import ExitStack
    ctx = ExitStack()
    with ctx:
        consts = ctx.enter_context(tc.tile_pool(name="consts", bufs=1))
        wpool = ctx.enter_context(tc.tile_pool(name="wpool", bufs=4))
        big = ctx.enter_context(tc.tile_pool(name="big", bufs=1))
        med = ctx.enter_context(tc.tile_pool(name="med", bufs=1))
        small = ctx.enter_context(tc.tile_pool(name="small", bufs=2))
        ps_proj = ctx.enter_context(
            tc.tile_pool(name="ps_proj", bufs=4, space="PSUM"))
        ps_half = ctx.enter_context(
            tc.tile_pool(name="ps_half", bufs=4, space="PSUM"))

        onesf_t = consts.tile([1, 128], F32R, tag="onesf")
        nc.sync.dma_start(onesf_t[:], onesf_d[:])
        bias_sb = {}
        for n, d in b_d.items():
            t = consts.tile([128, NCH], F32, tag="b_" + n)
            nc.sync.dma_start(t[:], d.rearrange("(c p) -> p c", p=128))
            bias_sb[n] = t
        bvb = consts.tile([128, E], BF16, tag="bvb")
        bv_ap = bv_d[:, :]
        nc.gpsimd.dma_start(
            out=bvb[:],
            in_=bass.AP(tensor=bv_ap.tensor, offset=bv_ap.offset,
                        ap=[[0, 128], [1, E]]))
        def _bc2(ap, n1, n2):
            """Broadcast a [P, Q] AP to [P, n1, n2, Q] with step-0 dims."""
            lst = [list(p) for p in ap.ap]
            return bass.AP(tensor=ap.tensor, offset=ap.offset,
                           ap=[lst[0], [0, n1], [0, n2]] + lst[1:])

        def load_wq(name, qo):
            """Quarter qo (out-cols qo*256..+256) of a single [E,E] weight."""
            t = wpool.tile([128, NCH, 256], BF16, tag="w", name=f"{name}_{qo}")
            nc.sync.dma_start(
                t[:],
                w_d[name][:, qo * 256:(qo + 1) * 256]
                .rearrange("(c p) o -> p c o", p=128))
            return t

        def load_w8(name, mo):
            """Eighth mo (out-cols mo*128..+128) of a [2E,E] GRU cat weight."""
            t = wpool.tile([128, 2 * NCH, 128], BF16, tag="w",
                           name=f"{name}_{mo}")
            nc.sync.dma_start(
                t[:],
                w_d[name][:, mo * 128:(mo + 1) * 128]
                .rearrange("(c p) o -> p c o", p=128))
            return t

        NB = B_PER_CORE

        # ---- batch-concat inputs: qin (kin last QL), oriq (key last QL) ----
        qin = med.tile([128, NB, NCH, QL], BF16, tag="qin", bufs=1)
        oriq = med.tile([128, NB, NCH, QL], BF16, tag="oriq", bufs=1)
        for b in range(NB):
            nc.sync.dma_start(
                qin[:, b], kinT_d[b][:, L - QL:]
                .rearrange("(c p) t -> p c t", p=128))
            nc.sync.dma_start(
                oriq[:, b], keyT_d[b][:, L - QL:]
                .rearrange("(c p) t -> p c t", p=128))

        # ---- q projection (both batches, block-diag head-pair layout) ----
        # qTz[p, b, g, hi, t]: quadrant (0:64, hi=0) = even head, (64:128,
        # hi=1) = odd head of pair g; opposite quadrants are zero so the
        # scores matmul can contract all 128 partitions.
        qTz = med.tile([128, NB, NCH, 2, QL], BF16, tag="qTz", bufs=1)
        nc.vector.memset(qTz[64:128, :, :, 0, :], 0.0)
        nc.vector.memset(qTz[0:64, :, :, 1, :], 0.0)
        wq_q = [load_wq("wq", qo) for qo in range(4)]
        for mo in range(NCH):
            p = ps_proj.tile([128, NB, QL], F32, tag="proj")
            wt = wq_q[mo // 2]
            for ci in range(NCH):
                nc.tensor.matmul(
                    p[:].rearrange("p a b -> p (a b)"),
                    wt[:, ci, (mo % 2) * 128:(mo % 2) * 128 + 128],
                    qin[:, :, ci, :],
                    start=(ci == 0), stop=(ci == NCH - 1))
            nc.scalar.activation(qTz[0:64, :, mo, 0, :], p[0:64], AF.Identity,
                                 bias=bias_sb["bq"][0:64, mo:mo + 1])
            nc.scalar.activation(qTz[64:128, :, mo, 1, :], p[64:128],
                                 AF.Identity,
                                 bias=bias_sb["bq"][64:128, mo:mo + 1])

        aoutT = med.tile([128, NB, NCH, QL], BF16, tag="aoutT", bufs=1)

        # ================= attention (per batch) =================
        for b in range(NB):
            kin = [big.tile([128, NCH, 512], BF16, tag="kin", bufs=2,
                            name=f"kin{b}h{n}") for n in range(2)]
            for n in range(2):
                nc.sync.dma_start(
                    kin[n][:],
                    kinT_d[b][:, n * 512:(n + 1) * 512]
                    .rearrange("(c p) t -> p c t", p=128))
            maskt = big.tile([128, NCH, QL], BF16, tag="mask", bufs=2,
                             name=f"mask{b}")
            nc.sync.dma_start(maskt[:],
                              maskT_d[b].rearrange("(c p) t -> p c t", p=128))

            # ---- k projection (streamed to DRAM scratch); n-inner loop so
            # each weight chunk's LDWEIGHTS is reused for 2 matmuls ----
            wk_q = [load_wq("wk", qo) for qo in range(4)]
            for mo in range(NCH):
                wt = wk_q[mo // 2]
                pk = [ps_proj.tile([128, 512], F32, tag="proj",
                                   name=f"pk{n}") for n in range(2)]
                for ci in range(NCH):
                    for n in range(2):
                        nc.tensor.matmul(
                            pk[n][:],
                            wt[:, ci, (mo % 2) * 128:(mo % 2) * 128 + 128],
                            kin[n][:, ci, :],
                            start=(ci == 0), stop=(ci == NCH - 1))
                for n in range(2):
                    kt = small.tile([128, 512], BF16, tag="ktmp", bufs=2)
                    nc.scalar.activation(kt[:], pk[n][:], AF.Identity,
                                         bias=bias_sb["bk"][:, mo:mo + 1])
                    nc.sync.dma_start(kts_d[b, mo, :, n * 512:(n + 1) * 512],
                                      kt[:])

            # ---- v projection (token-major, 65th column = ones for the
            # softmax row-sum); ci-outer loop keeps keyc chunk stationary
            # in the PE across the 4 output quarters ----
            wv_q = [load_wq("wv", qo) for qo in range(4)]
            vkm = big.tile([128, NCH, 2 * NCH, 65], BF16, tag="vkm", bufs=2,
                           name=f"vkm{b}")
            nc.vector.memset(vkm[:, :, :, 64:65], 1.0)
            for kc in range(NCH):
                keyc = small.tile([128, NCH, 128], BF16, tag="keyc", bufs=2)
                nc.sync.dma_start(
                    keyc[:],
                    keyT_d[b][:, kc * 128:(kc + 1) * 128]
                    .rearrange("(c p) t -> p c t", p=128))
                for qp in range(2):
                    pv = [ps_half.tile([128, 256], F32, tag="half",
                                       name=f"pv{j}") for j in range(2)]
                    for ci in range(NCH):
                        for j in range(2):
                            nc.tensor.matmul(
                                pv[j][:], keyc[:, ci, :],
                                wv_q[2 * qp + j][:, ci, :],
                                start=(ci == 0), stop=(ci == NCH - 1))
                    for j in range(2):
                        q4 = 2 * qp + j
                        nc.vector.tensor_add(
                            vkm[:, kc, 4 * q4:4 * q4 + 4, 0:64],
                            pv[j][:].rearrange("p (h d) -> p h d", h=4),
                            bvb[:, q4 * 256:(q4 + 1) * 256]
                            .rearrange("p (h d) -> p h d", h=4))

            # ---- attention, per 2-head group g ----
            awT = med.tile([128, NCH, QL], F32, tag="awT", bufs=1,
                           name=f"awT{b}")
            # pins guard the pT slot two generations back (pT bufs=2)
            pin_pe = [None, None]   # last attn@V matmul per parity
            pin_dve = [None, None]  # last awtmp mul per parity
            for g in range(NCH):
                ktd = med.tile([128, L], BF16, tag="ktd", bufs=2,
                               name=f"ktd{b}_{g}")
                nc.sync.dma_start(ktd[:], kts_d[b, g])
                pT = med.tile([128, NCH, 2, QL], BF16, tag="pT", bufs=2,
                              name=f"pT{b}_{g}")
                for kc in range(NCH):
                    ps = ps_proj.tile([128, 2, QL], F32, tag="proj")
                    nc.tensor.matmul(
                        ps[:].rearrange("p a b -> p (a b)"),
                        ktd[:, kc * 128:(kc + 1) * 128],
                        qTz[:, b, g].rearrange("p a b -> p (a b)"),
                        start=True, stop=True)
                    nc.scalar.activation(pT[:, kc], ps[:], AF.Exp, scale=0.125)
                # one batched mask multiply for all kc chunks (hi broadcast)
                mi = nc.vector.tensor_mul(pT[:], pT[:],
                                          _bc_hi2(maskt[:, :, :]))
                if pin_dve[g % 2] is not None:
                    add_dep_helper(mi.ins, pin_dve[g % 2].ins, sync=False,
                                   reason="order pT DVE readers across groups")

                # attn @ V per head (M=65; row 64 = softmax denominator)
                pav = [ps_half.tile([65, QL], F32, tag="half",
                                    name=f"pav{hi}") for hi in range(2)]
                for kc in range(NCH):
                    for hi in range(2):
                        av = nc.tensor.matmul(
                            pav[hi][:, :],
                            vkm[:, kc, 2 * g + hi, :],
                            pT[:, kc, hi, :],
                            start=(kc == 0), stop=(kc == NCH - 1))
                        if kc == 0 and hi == 0 and pin_pe[g % 2] is not None:
                            add_dep_helper(av.ins, pin_pe[g % 2].ins, sync=False,
                                           reason="order pT PE readers across groups")
                pin_pe[g % 2] = av

                # row-sums sit on partition 64 (DVE lanes can't cross
                # partitions): copy to SBUF at partition 64, reciprocal in
                # place, broadcast via f32r ones-matmul whose lhsT also
                # lives at partition 64
                rs = small.tile([65, 2, QL], F32, tag="rs", bufs=1)
                nc.vector.tensor_copy(rs[64:65, 0, :], pav[0][64:65, :])
                nc.vector.tensor_copy(rs[64:65, 1, :], pav[1][64:65, :])
                rs0 = small.tile([1, 2, QL], F32, tag="rs0", bufs=1)
                nc.sync.dma_start(rs0[:], rs[64:65])
                nc.vector.reciprocal_approx_fast(rs0[:], rs0[:])
                r1 = small.tile([1, 2, QL], F32R, tag="r1", bufs=1)
                nc.vector.tensor_copy(r1[:], rs0[:])
                rbp = ps_proj.tile([128, 2, QL], F32, tag="proj")
                nc.tensor.matmul(rbp[:].rearrange("p a b -> p (a b)"),
                                 onesf_t[:],
                                 r1[:].rearrange("p a b -> p (a b)"),
                                 start=True, stop=True)
                recipb = small.tile([64, 2, QL], F32, tag="recipb", bufs=1)
                nc.vector.tensor_copy(recipb[:], rbp[0:64])
                # 1/H folded in here: rbpb feeds only the attn_w path
                rbpb = small.tile([128, 2, QL], BF16, tag="rbpb", bufs=1)
                nc.scalar.activation(rbpb[:], rbp[:], AF.Copy, scale=1.0 / H)

                # normalize attn@V during eviction; odd head partition-shifted
                nc.vector.tensor_mul(aoutT[0:64, b, g, :], pav[0][0:64, :],
                                     recipb[:, 0, :])
                sh = small.tile([64, QL], BF16, tag="btmp")
                nc.vector.tensor_mul(sh[:, :], pav[1][0:64, :],
                                     recipb[:, 1, :])
                nc.sync.dma_start(aoutT[64:128, b, g, :], sh[:, :])

                # attn_w: fused normalize-mul, hi-pair add on DVE, then one
                # serial chain add per group on GpSimd
                tmp = med.tile([128, NCH, 2, QL], BF16, tag="awtmp", bufs=1)
                lm = nc.vector.tensor_mul(tmp[:], pT[:], _hbcast(rbpb[:], NCH))
                pin_dve[g % 2] = lm
                tg = med.tile([128, NCH, QL], BF16, tag="tg", bufs=2,
                              name=f"tg{g % 2}")
                nc.vector.tensor_add(tg[:], tmp[:, :, 0, :], tmp[:, :, 1, :])
                if g == 0:
                    nc.gpsimd.tensor_copy(awT[:], tg[:])
                else:
                    nc.gpsimd.tensor_add(awT[:], awT[:], tg[:])

            nc.sync.dma_start(
                attnwT_d[b].rearrange("(c p) t -> p c t", p=128), awT[:])

        # ================= out proj + GRU (batch-concat) =================
        wo_q = [load_wq("wo", qo) for qo in range(4)]
        outT = med.tile([128, NB, NCH, QL], BF16, tag="outT", bufs=1)
        for mo in range(NCH):
            p = ps_proj.tile([128, NB, QL], F32, tag="proj")
            wt = wo_q[mo // 2]
            for ci in range(NCH):
                nc.tensor.matmul(
                    p[:].rearrange("p a b -> p (a b)"),
                    wt[:, ci, (mo % 2) * 128:(mo % 2) * 128 + 128],
                    aoutT[:, :, ci, :], start=(ci == 0), stop=(ci == NCH - 1))
            t = small.tile([128, NB, QL], F32, tag="btmp2")
            nc.scalar.activation(t[:], p[:], AF.Relu,
                                 bias=bias_sb["bo"][:, mo:mo + 1])
            nc.vector.tensor_add(outT[:, :, mo, :], t[:], oriq[:, :, mo, :])

        # r-gate: rq = relu([oriq;outT] @ wrC + brz) * oriq
        rqT = med.tile([128, NB, NCH, QL], BF16, tag="rqT", bufs=1)
        hT = med.tile([128, NB, NCH, QL], BF16, tag="hT", bufs=1)

        for stage, (wname, bias, func) in enumerate(
                [("wrC", "brz", AF.Relu), ("wgC", "bgg", AF.Tanh),
                 ("wzC", "bzz", AF.Relu)]):
            xside = rqT if stage == 1 else oriq
            for mo in range(NCH):
                w8 = load_w8(wname, mo)
                p = ps_proj.tile([128, NB, QL], F32, tag="proj")
                for ci in range(2 * NCH):
                    rhs = (xside[:, :, ci, :] if ci < NCH
                           else outT[:, :, ci - NCH, :])
                    nc.tensor.matmul(
                        p[:].rearrange("p a b -> p (a b)"), w8[:, ci, :], rhs,
                        start=(ci == 0), stop=(ci == 2 * NCH - 1))
                if stage == 0:   # r -> rq
                    t = small.tile([128, NB, QL], F32, tag="btmp2")
                    nc.scalar.activation(t[:], p[:], func,
                                         bias=bias_sb[bias][:, mo:mo + 1])
                    nc.vector.tensor_mul(rqT[:, :, mo, :], t[:],
                                         oriq[:, :, mo, :])
                elif stage == 1:  # h
                    nc.scalar.activation(hT[:, :, mo, :], p[:], func,
                                         bias=bias_sb[bias][:, mo:mo + 1])
                else:            # z + final blend + store (blend on DVE —
                    # it idles during the matmul-heavy GRU phase)
                    zt = small.tile([128, NB, QL], F32, tag="btmp2")
                    nc.scalar.activation(zt[:], p[:], func,
                                         bias=bias_sb[bias][:, mo:mo + 1])
                    d = small.tile([128, NB, QL], F32, tag="btmp2")
                    nc.vector.tensor_sub(d[:], hT[:, :, mo, :],
                                         oriq[:, :, mo, :])
                    nc.vector.tensor_mul(d[:], d[:], zt[:])
                    fin = small.tile([128, NB, QL], F32, tag="btmp2")
                    nc.vector.tensor_add(fin[:], d[:], oriq[:, :, mo, :])
                    for b in range(NB):
                        nc.sync.dma_start(
                            outT_d[b][mo * 128:(mo + 1) * 128, :], fin[:, b, :])


def prep_inputs_core(core, key, pe, key_index, key_padding_mask,
                     in_proj_w, in_proj_b, out_w, out_b, gw, gb):
    b0 = core * B_PER_CORE
    sl = slice(b0, b0 + B_PER_CORE)
    keyc = np.asarray(key[sl], np.float32)
    kin = keyc + np.asarray(pe[sl], np.float32)
    kinT = np.ascontiguousarray(kin.transpose(0, 2, 1))
    keyT = np.ascontiguousarray(keyc.transpose(0, 2, 1))

    ki = np.asarray(key_index[sl])
    pad = np.asarray(key_padding_mask[sl])
    qi = ki[:, L - QL:]
    ri = ki[:, :L - QL]
    import ml_dtypes
    bf16 = ml_dtypes.bfloat16
    allowed = np.zeros((B_PER_CORE, L, QL), np.float32)
    allowed[:, :L - QL, :] = ((ri[:, :, None] < qi[:, None, :])
                              & ~pad[:, :L - QL, None])
    allowed[:, L - QL:, :] = np.eye(QL, dtype=np.float32)[None]

    w32 = lambda x: np.asarray(x, np.float32)
    wbf = lambda x: np.ascontiguousarray(x).astype(bf16)
    im = {
        "kinT": wbf(kinT), "keyT": wbf(keyT),
        "maskT": allowed.astype(bf16),
        "ones": np.ones((128, 128), bf16),
        "onesf": np.ones((1, 128), np.float32),
        "bv": w32(in_proj_b[2 * E:]).reshape(1, E),
        "bq": w32(in_proj_b[:E]),
        "bk": w32(in_proj_b[E:2 * E]),
        "bo": w32(out_b),
        "brz": w32(gb["bxr"] + gb["byr"]),
        "bzz": w32(gb["bxz"] + gb["byz"]),
        "bgg": w32(gb["bxg"] + gb["byg"]),
        "wqT": wbf(w32(in_proj_w[:E]).T),
        "wkT": wbf(w32(in_proj_w[E:2 * E]).T),
        "wvT": wbf(w32(in_proj_w[2 * E:]).T),
        "woT": wbf(w32(out_w).T),
        "wrC": wbf(np.concatenate([w32(gw["wxr"]).T, w32(gw["wyr"]).T], 0)),
        "wzC": wbf(np.concatenate([w32(gw["wxz"]).T, w32(gw["wyz"]).T], 0)),
        "wgC": wbf(np.concatenate([w32(gw["wxg"]).T, w32(gw["wyg"]).T], 0)),
    }
    return im


def postprocess(results):
    outs, aws = [], []
    for r in results:
        outs.append(r["outT"].transpose(0, 2, 1))
        aws.append(r["attnwT"].transpose(0, 2, 1))
    return (np.concatenate(outs, 0), np.concatenate(aws, 0))


_NC_CACHE = {}


def kernel(key, pe, key_index, key_padding_mask, query_length,
           in_proj_w, in_proj_b, out_w, out_b,
           wxr, bxr, wyr, byr, wxz, bxz, wyz, byz, wxg, bxg, wyg, byg):
    """Full-input entry point: shard B=16 across 8 NeuronCores, run, gather."""
    from concourse.bass_utils import run_bass_kernel_spmd

    key = np.asarray(key)
    assert int(query_length) == QL and key.shape == (16, L, E)
    if "nc" not in _NC_CACHE:
        _NC_CACHE["nc"] = build_kernel(num_devices=8)
    nc = _NC_CACHE["nc"]

    gw = {"wxr": wxr, "wyr": wyr, "wxz": wxz, "wyz": wyz,
          "wxg": wxg, "wyg": wyg}
    gb = {"bxr": bxr, "byr": byr, "bxz": bxz, "byz": byz,
          "bxg": bxg, "byg": byg}
    in_maps = [prep_inputs_core(c, key, pe, key_index, key_padding_mask,
                                in_proj_w, in_proj_b, out_w, out_b, gw, gb)
               for c in range(8)]
    res = run_bass_kernel_spmd(nc, in_maps, core_ids=list(range(8)))
    out, attn_w = postprocess(res.results)
    return out.astype(np.float32), attn_w.astype(np.float32)



# revision 17
# speedup vs baseline: 1.2272x; 1.2123x over previous
"""Bass/Tile kernel for EpisodeMultiheadAttentionBlock on TRN2.

Per-core: 2 batch elements (data-parallel over B=16 across 8 cores).

Layout: activations feature-major [feature parts, token free]; V token-major.
All matmuls float32r. Softmax without max-subtraction; multiplicative 0/1
mask after exp; 1/rowsum via ones-matmul (which also broadcasts it across
all partitions); pT normalized in-place so attn@V and attn_w both consume
normalized attention weights.
"""

import os

import numpy as np

import concourse.bass as bass
import concourse.mybir as mybir
import concourse.tile as tile
from concourse import bacc
from concourse.tile_rust import add_dep_helper

F32 = mybir.dt.float32
F32R = mybir.dt.float32r
BF16 = mybir.dt.bfloat16
AF = mybir.ActivationFunctionType

B_PER_CORE = 2
L = 1024
E = 1024
H = 16
QL = 256
NCH = 8

SINGLES = ["wq", "wk", "wv", "wo"]
GRUCATS = ["wrC", "wzC", "wgC"]


def build_kernel(num_devices=8):
    nc = bacc.Bacc("TRN2", target_bir_lowering=False, debug=False,
                   num_devices=num_devices)

    kinT_d = nc.dram_tensor("kinT", [B_PER_CORE, E, L], BF16, kind="ExternalInput")
    keyT_d = nc.dram_tensor("keyT", [B_PER_CORE, E, L], BF16, kind="ExternalInput")
    maskT_d = nc.dram_tensor("maskT", [B_PER_CORE, L, QL], mybir.dt.bfloat16, kind="ExternalInput")
    ones_d = nc.dram_tensor("ones", [128, 128], BF16, kind="ExternalInput")
    onesf_d = nc.dram_tensor("onesf", [1, 128], F32R, kind="ExternalInput")
    w_d = {n: nc.dram_tensor(n + "T", [E, E], BF16, kind="ExternalInput")
           for n in SINGLES}
    for n in GRUCATS:
        w_d[n] = nc.dram_tensor(n, [2 * E, E], BF16, kind="ExternalInput")
    b_d = {n: nc.dram_tensor(n, [E], F32, kind="ExternalInput")
           for n in ["bq", "bk", "bo", "brz", "bzz", "bgg"]}
    bv_d = nc.dram_tensor("bv", [1, E], F32, kind="ExternalInput")

    kts_d = nc.dram_tensor("kts", [B_PER_CORE, NCH, 128, L], BF16)
    outT_d = nc.dram_tensor("outT", [B_PER_CORE, E, QL], F32, kind="ExternalOutput")
    attnwT_d = nc.dram_tensor("attnwT", [B_PER_CORE, L, QL], F32,
                              kind="ExternalOutput")

    with tile.TileContext(nc) as tc, nc.allow_low_precision(
            reason="float32r tiles feed f32r matmuls; PSUM accumulation is f32"):
        _body(nc, tc, kinT_d, keyT_d, maskT_d, ones_d, onesf_d,
              w_d, b_d, bv_d, kts_d, outT_d, attnwT_d)
    nc.compile()
    return nc


def _hbcast(ap, n):
    """Broadcast a [P, Q] AP to [P, n, Q] with a step-0 middle dim."""
    lst = [list(p) for p in ap.ap]
    return bass.AP(tensor=ap.tensor, offset=ap.offset,
                   ap=[lst[0], [0, n]] + lst[1:])


def _bc_hi2(ap):
    """Broadcast a [P, C, Q] AP to [P, C, 2, Q] with a step-0 hi dim."""
    lst = [list(p) for p in ap.ap]
    return bass.AP(tensor=ap.tensor, offset=ap.offset,
                   ap=[lst[0], lst[1], [0, 2]] + lst[2:])


def _body(nc, tc, kinT_d, keyT_d, maskT_d, ones_d, onesf_d,
          w_d, b_d, bv_d, kts_d, outT_d, attnwT_d):
    from contextlib import ExitStack
    ctx = ExitStack()
    with ctx:
        consts = ctx.enter_context(tc.tile_pool(name="consts", bufs=1))
        wpool = ctx.enter_context(tc.tile_pool(name="wpool", bufs=6))
        big = ctx.enter_context(tc.tile_pool(name="big", bufs=1))
        med = ctx.enter_context(tc.tile_pool(name="med", bufs=1))
        small = ctx.enter_context(tc.tile_pool(name="small", bufs=2))
        ps_proj = ctx.enter_context(
            tc.tile_pool(name="ps_proj", bufs=4, space="PSUM"))
        ps_half = ctx.enter_context(
            tc.tile_pool(name="ps_half", bufs=4, space="PSUM"))

        onesf_t = consts.tile([1, 128], F32R, tag="onesf")
        nc.sync.dma_start(onesf_t[:], onesf_d[:])
        bias_sb = {}
        for n, d in b_d.items():
            t = consts.tile([128, NCH], F32, tag="b_" + n)
            nc.sync.dma_start(t[:], d.rearrange("(c p) -> p c", p=128))
            bias_sb[n] = t
        bvb = consts.tile([128, E], BF16, tag="bvb")
        bv_ap = bv_d[:, :]
        nc.gpsimd.dma_start(
            out=bvb[:],
            in_=bass.AP(tensor=bv_ap.tensor, offset=bv_ap.offset,
                        ap=[[0, 128], [1, E]]))
        def _bc2(ap, n1, n2):
            """Broadcast a [P, Q] AP to [P, n1, n2, Q] with step-0 dims."""
            lst = [list(p) for p in ap.ap]
            return bass.AP(tensor=ap.tensor, offset=ap.offset,
                           ap=[lst[0], [0, n1], [0, n2]] + lst[1:])

        def load_wq(name, qo):
            """Quarter qo (out-cols qo*256..+256) of a single [E,E] weight."""
            t = wpool.tile([128, NCH, 256], BF16, tag="w", name=f"{name}_{qo}")
            nc.sync.dma_start(
                t[:],
                w_d[name][:, qo * 256:(qo + 1) * 256]
                .rearrange("(c p) o -> p c o", p=128))
            return t

        def load_w8(name, mo):
            """Eighth mo (out-cols mo*128..+128) of a [2E,E] GRU cat weight."""
            t = wpool.tile([128, 2 * NCH, 128], BF16, tag="w",
                           name=f"{name}_{mo}")
            nc.sync.dma_start(
                t[:],
                w_d[name][:, mo * 128:(mo + 1) * 128]
                .rearrange("(c p) o -> p c o", p=128))
            return t

        NB = B_PER_CORE

        # ---- batch-concat inputs: qin (kin last QL), oriq (key last QL) ----
        qin = med.tile([128, NB, NCH, QL], BF16, tag="qin", bufs=1)
        oriq = med.tile([128, NB, NCH, QL], BF16, tag="oriq", bufs=1)
        for b in range(NB):
            nc.sync.dma_start(
                qin[:, b], kinT_d[b][:, L - QL:]
                .rearrange("(c p) t -> p c t", p=128))
            nc.sync.dma_start(
                oriq[:, b], keyT_d[b][:, L - QL:]
                .rearrange("(c p) t -> p c t", p=128))

        # ---- q projection (both batches, block-diag head-pair layout) ----
        # qTz[p, b, g, hi, t]: quadrant (0:64, hi=0) = even head, (64:128,
        # hi=1) = odd head of pair g; opposite quadrants are zero so the
        # scores matmul can contract all 128 partitions.
        qTz = med.tile([128, NB, NCH, 2, QL], BF16, tag="qTz", bufs=1)
        nc.vector.memset(qTz[64:128, :, :, 0, :], 0.0)
        nc.vector.memset(qTz[0:64, :, :, 1, :], 0.0)
        wq_q = [load_wq("wq", qo) for qo in range(4)]
        for mo in range(NCH):
            p = ps_proj.tile([128, NB, QL], F32, tag="proj")
            wt = wq_q[mo // 2]
            for ci in range(NCH):
                nc.tensor.matmul(
                    p[:].rearrange("p a b -> p (a b)"),
                    wt[:, ci, (mo % 2) * 128:(mo % 2) * 128 + 128],
                    qin[:, :, ci, :],
                    start=(ci == 0), stop=(ci == NCH - 1))
            nc.scalar.activation(qTz[0:64, :, mo, 0, :], p[0:64], AF.Identity,
                                 bias=bias_sb["bq"][0:64, mo:mo + 1])
            nc.scalar.activation(qTz[64:128, :, mo, 1, :], p[64:128],
                                 AF.Identity,
                                 bias=bias_sb["bq"][64:128, mo:mo + 1])

        aoutT = med.tile([128, NB, NCH, QL], BF16, tag="aoutT", bufs=1)

        # ================= attention (per batch) =================
        for b in range(NB):
            kin = [big.tile([128, NCH, 512], BF16, tag="kin", bufs=2,
                            name=f"kin{b}h{n}") for n in range(2)]
            for n in range(2):
                nc.sync.dma_start(
                    kin[n][:],
                    kinT_d[b][:, n * 512:(n + 1) * 512]
                    .rearrange("(c p) t -> p c t", p=128))
            maskt = big.tile([128, NCH, QL], BF16, tag="mask", bufs=2,
                             name=f"mask{b}")
            nc.sync.dma_start(maskt[:],
                              maskT_d[b].rearrange("(c p) t -> p c t", p=128))

            # ---- k projection (streamed to DRAM scratch); n-inner loop so
            # each weight chunk's LDWEIGHTS is reused for 2 matmuls ----
            wk_q = [load_wq("wk", qo) for qo in range(4)]
            for mo in range(NCH):
                wt = wk_q[mo // 2]
                pk = [ps_proj.tile([128, 512], F32, tag="proj",
                                   name=f"pk{n}") for n in range(2)]
                for ci in range(NCH):
                    for n in range(2):
                        nc.tensor.matmul(
                            pk[n][:],
                            wt[:, ci, (mo % 2) * 128:(mo % 2) * 128 + 128],
                            kin[n][:, ci, :],
                            start=(ci == 0), stop=(ci == NCH - 1))
                for n in range(2):
                    kt = small.tile([128, 512], BF16, tag="ktmp", bufs=2)
                    nc.scalar.activation(kt[:], pk[n][:], AF.Identity,
                                         bias=bias_sb["bk"][:, mo:mo + 1])
                    nc.sync.dma_start(kts_d[b, mo, :, n * 512:(n + 1) * 512],
                                      kt[:])

            # ---- v projection (token-major, 65th column = ones for the
            # softmax row-sum); ci-outer loop keeps keyc chunk stationary
            # in the PE across the 4 output quarters ----
            wv_q = [load_wq("wv", qo) for qo in range(4)]
            vkm = big.tile([128, NCH, 2 * NCH, 65], BF16, tag="vkm", bufs=2,
                           name=f"vkm{b}")
            nc.vector.memset(vkm[:, :, :, 64:65], 1.0)
            for kc in range(NCH):
                keyc = small.tile([128, NCH, 128], BF16, tag="keyc", bufs=2)
                nc.sync.dma_start(
                    keyc[:],
                    keyT_d[b][:, kc * 128:(kc + 1) * 128]
                    .rearrange("(c p) t -> p c t", p=128))
                for qp in range(2):
                    pv = [ps_half.tile([128, 256], F32, tag="half",
                                       name=f"pv{j}") for j in range(2)]
                    for ci in range(NCH):
                        for j in range(2):
                            nc.tensor.matmul(
                                pv[j][:], keyc[:, ci, :],
                                wv_q[2 * qp + j][:, ci, :],
                                start=(ci == 0), stop=(ci == NCH - 1))
                    for j in range(2):
                        q4 = 2 * qp + j
                        nc.vector.tensor_add(
                            vkm[:, kc, 4 * q4:4 * q4 + 4, 0:64],
                            pv[j][:].rearrange("p (h d) -> p h d", h=4),
                            bvb[:, q4 * 256:(q4 + 1) * 256]
                            .rearrange("p (h d) -> p h d", h=4))

            # ---- attention, per 2-head group g ----
            awT = med.tile([128, NCH, QL], F32, tag="awT", bufs=1,
                           name=f"awT{b}")
            # pins guard the pT slot two generations back (pT bufs=2)
            pin_pe = [None, None]   # last attn@V matmul per parity
            pin_dve = [None, None]  # last awtmp mul per parity
            for g in range(NCH):
                ktd = med.tile([128, L], BF16, tag="ktd", bufs=2,
                               name=f"ktd{b}_{g}")
                nc.sync.dma_start(ktd[:], kts_d[b, g])
                pT = med.tile([128, NCH, 2, QL], BF16, tag="pT", bufs=2,
                              name=f"pT{b}_{g}")
                for kc in range(NCH):
                    ps = ps_proj.tile([128, 2, QL], F32, tag="proj")
                    nc.tensor.matmul(
                        ps[:].rearrange("p a b -> p (a b)"),
                        ktd[:, kc * 128:(kc + 1) * 128],
                        qTz[:, b, g].rearrange("p a b -> p (a b)"),
                        start=True, stop=True)
                    nc.scalar.activation(pT[:, kc], ps[:], AF.Exp, scale=0.125)
                # one batched mask multiply for all kc chunks (hi broadcast)
                mi = nc.vector.tensor_mul(pT[:], pT[:],
                                          _bc_hi2(maskt[:, :, :]))
                if pin_dve[g % 2] is not None:
                    add_dep_helper(mi.ins, pin_dve[g % 2].ins, sync=False,
                                   reason="order pT DVE readers across groups")

                # attn @ V per head (M=65; row 64 = softmax denominator)
                pav = [ps_half.tile([65, QL], F32, tag="half",
                                    name=f"pav{hi}") for hi in range(2)]
                for kc in range(NCH):
                    for hi in range(2):
                        av = nc.tensor.matmul(
                            pav[hi][:, :],
                            vkm[:, kc, 2 * g + hi, :],
                            pT[:, kc, hi, :],
                            start=(kc == 0), stop=(kc == NCH - 1))
                        if kc == 0 and hi == 0 and pin_pe[g % 2] is not None:
                            add_dep_helper(av.ins, pin_pe[g % 2].ins, sync=False,
                                           reason="order pT PE readers across groups")
                pin_pe[g % 2] = av

                # row-sums sit on partition 64 (DVE lanes can't cross
                # partitions): copy to SBUF at partition 64, reciprocal in
                # place, broadcast via f32r ones-matmul whose lhsT also
                # lives at partition 64
                rs = small.tile([65, 2, QL], F32, tag="rs", bufs=1)
                nc.vector.tensor_copy(rs[64:65, 0, :], pav[0][64:65, :])
                nc.vector.tensor_copy(rs[64:65, 1, :], pav[1][64:65, :])
                rs0 = small.tile([1, 2, QL], F32, tag="rs0", bufs=1)
                nc.sync.dma_start(rs0[:], rs[64:65])
                nc.vector.reciprocal_approx_fast(rs0[:], rs0[:])
                r1 = small.tile([1, 2, QL], F32R, tag="r1", bufs=1)
                nc.vector.tensor_copy(r1[:], rs0[:])
                rbp = ps_proj.tile([128, 2, QL], F32, tag="proj")
                nc.tensor.matmul(rbp[:].rearrange("p a b -> p (a b)"),
                                 onesf_t[:],
                                 r1[:].rearrange("p a b -> p (a b)"),
                                 start=True, stop=True)
                recipb = small.tile([64, 2, QL], F32, tag="recipb", bufs=1)
                nc.vector.tensor_copy(recipb[:], rbp[0:64])
                # 1/H folded in here: rbpb feeds only the attn_w path
                rbpb = small.tile([128, 2, QL], BF16, tag="rbpb", bufs=1)
                nc.scalar.activation(rbpb[:], rbp[:], AF.Copy, scale=1.0 / H)

                # normalize attn@V during eviction; odd head partition-shifted
                nc.vector.tensor_mul(aoutT[0:64, b, g, :], pav[0][0:64, :],
                                     recipb[:, 0, :])
                sh = small.tile([64, QL], BF16, tag="btmp")
                nc.vector.tensor_mul(sh[:, :], pav[1][0:64, :],
                                     recipb[:, 1, :])
                nc.sync.dma_start(aoutT[64:128, b, g, :], sh[:, :])

                # attn_w: fused normalize-mul, hi-pair add on DVE, then one
                # serial chain add per group on GpSimd
                tmp = med.tile([128, NCH, 2, QL], BF16, tag="awtmp", bufs=1)
                lm = nc.vector.tensor_mul(tmp[:], pT[:], _hbcast(rbpb[:], NCH))
                pin_dve[g % 2] = lm
                tg = med.tile([128, NCH, QL], BF16, tag="tg", bufs=2,
                              name=f"tg{g % 2}")
                nc.vector.tensor_add(tg[:], tmp[:, :, 0, :], tmp[:, :, 1, :])
                if g == 0:
                    nc.gpsimd.tensor_copy(awT[:], tg[:])
                else:
                    nc.gpsimd.tensor_add(awT[:], awT[:], tg[:])

            nc.sync.dma_start(
                attnwT_d[b].rearrange("(c p) t -> p c t", p=128), awT[:])

        # ================= out proj + GRU (batch-concat) =================
        wo_q = [load_wq("wo", qo) for qo in range(4)]
        outT = med.tile([128, NB, NCH, QL], BF16, tag="outT", bufs=1)
        for mo in range(NCH):
            p = ps_proj.tile([128, NB, QL], F32, tag="proj")
            wt = wo_q[mo // 2]
            for ci in range(NCH):
                nc.tensor.matmul(
                    p[:].rearrange("p a b -> p (a b)"),
                    wt[:, ci, (mo % 2) * 128:(mo % 2) * 128 + 128],
                    aoutT[:, :, ci, :], start=(ci == 0), stop=(ci == NCH - 1))
            t = small.tile([128, NB, QL], F32, tag="btmp2")
            nc.scalar.activation(t[:], p[:], AF.Relu,
                                 bias=bias_sb["bo"][:, mo:mo + 1])
            nc.vector.tensor_add(outT[:, :, mo, :], t[:], oriq[:, :, mo, :])

        # r-gate: rq = relu([oriq;outT] @ wrC + brz) * oriq
        rqT = med.tile([128, NB, NCH, QL], BF16, tag="rqT", bufs=1)
        hT = med.tile([128, NB, NCH, QL], BF16, tag="hT", bufs=1)

        for stage, (wname, bias, func) in enumerate(
                [("wrC", "brz", AF.Relu), ("wgC", "bgg", AF.Tanh),
                 ("wzC", "bzz", AF.Relu)]):
            xside = rqT if stage == 1 else oriq
            for mo in range(NCH):
                w8 = load_w8(wname, mo)
                p = ps_proj.tile([128, NB, QL], F32, tag="proj")
                for ci in range(2 * NCH):
                    rhs = (xside[:, :, ci, :] if ci < NCH
                           else outT[:, :, ci - NCH, :])
                    nc.tensor.matmul(
                        p[:].rearrange("p a b -> p (a b)"), w8[:, ci, :], rhs,
                        start=(ci == 0), stop=(ci == 2 * NCH - 1))
                if stage == 0:   # r -> rq
                    t = small.tile([128, NB, QL], F32, tag="btmp2")
                    nc.scalar.activation(t[:], p[:], func,
                                         bias=bias_sb[bias][:, mo:mo + 1])
                    nc.vector.tensor_mul(rqT[:, :, mo, :], t[:],
                                         oriq[:, :, mo, :])
                elif stage == 1:  # h
                    nc.scalar.activation(hT[:, :, mo, :], p[:], func,
                                         bias=bias_sb[bias][:, mo:mo + 1])
                else:            # z + final blend + store (blend on DVE —
                    # it idles during the matmul-heavy GRU phase)
                    zt = small.tile([128, NB, QL], F32, tag="btmp2")
                    nc.scalar.activation(zt[:], p[:], func,
                                         bias=bias_sb[bias][:, mo:mo + 1])
                    d = small.tile([128, NB, QL], F32, tag="btmp2")
                    nc.vector.tensor_sub(d[:], hT[:, :, mo, :],
                                         oriq[:, :, mo, :])
                    nc.vector.tensor_mul(d[:], d[:], zt[:])
                    fin = small.tile([128, NB, QL], F32, tag="btmp2")
                    nc.vector.tensor_add(fin[:], d[:], oriq[:, :, mo, :])
                    for b in range(NB):
                        nc.sync.dma_start(
                            outT_d[b][mo * 128:(mo + 1) * 128, :], fin[:, b, :])


def prep_inputs_core(core, key, pe, key_index, key_padding_mask,
                     in_proj_w, in_proj_b, out_w, out_b, gw, gb):
    b0 = core * B_PER_CORE
    sl = slice(b0, b0 + B_PER_CORE)
    keyc = np.asarray(key[sl], np.float32)
    kin = keyc + np.asarray(pe[sl], np.float32)
    kinT = np.ascontiguousarray(kin.transpose(0, 2, 1))
    keyT = np.ascontiguousarray(keyc.transpose(0, 2, 1))

    ki = np.asarray(key_index[sl])
    pad = np.asarray(key_padding_mask[sl])
    qi = ki[:, L - QL:]
    ri = ki[:, :L - QL]
    import ml_dtypes
    bf16 = ml_dtypes.bfloat16
    allowed = np.zeros((B_PER_CORE, L, QL), np.float32)
    allowed[:, :L - QL, :] = ((ri[:, :, None] < qi[:, None, :])
                              & ~pad[:, :L - QL, None])
    allowed[:, L - QL:, :] = np.eye(QL, dtype=np.float32)[None]

    w32 = lambda x: np.asarray(x, np.float32)
    wbf = lambda x: np.ascontiguousarray(x).astype(bf16)
    im = {
        "kinT": wbf(kinT), "keyT": wbf(keyT),
        "maskT": allowed.astype(bf16),
        "ones": np.ones((128, 128), bf16),
        "onesf": np.ones((1, 128), np.float32),
        "bv": w32(in_proj_b[2 * E:]).reshape(1, E),
        "bq": w32(in_proj_b[:E]),
        "bk": w32(in_proj_b[E:2 * E]),
        "bo": w32(out_b),
        "brz": w32(gb["bxr"] + gb["byr"]),
        "bzz": w32(gb["bxz"] + gb["byz"]),
        "bgg": w32(gb["bxg"] + gb["byg"]),
        "wqT": wbf(w32(in_proj_w[:E]).T),
        "wkT": wbf(w32(in_proj_w[E:2 * E]).T),
        "wvT": wbf(w32(in_proj_w[2 * E:]).T),
        "woT": wbf(w32(out_w).T),
        "wrC": wbf(np.concatenate([w32(gw["wxr"]).T, w32(gw["wyr"]).T], 0)),
        "wzC": wbf(np.concatenate([w32(gw["wxz"]).T, w32(gw["wyz"]).T], 0)),
        "wgC": wbf(np.concatenate([w32(gw["wxg"]).T, w32(gw["wyg"]).T], 0)),
    }
    return im


def postprocess(results):
    outs, aws = [], []
    for r in results:
        outs.append(r["outT"].transpose(0, 2, 1))
        aws.append(r["attnwT"].transpose(0, 2, 1))
    return (np.concatenate(outs, 0), np.concatenate(aws, 0))


_NC_CACHE = {}


def kernel(key, pe, key_index, key_padding_mask, query_length,
           in_proj_w, in_proj_b, out_w, out_b,
           wxr, bxr, wyr, byr, wxz, bxz, wyz, byz, wxg, bxg, wyg, byg):
    """Full-input entry point: shard B=16 across 8 NeuronCores, run, gather."""
    from concourse.bass_utils import run_bass_kernel_spmd

    key = np.asarray(key)
    assert int(query_length) == QL and key.shape == (16, L, E)
    if "nc" not in _NC_CACHE:
        _NC_CACHE["nc"] = build_kernel(num_devices=8)
    nc = _NC_CACHE["nc"]

    gw = {"wxr": wxr, "wyr": wyr, "wxz": wxz, "wyz": wyz,
          "wxg": wxg, "wyg": wyg}
    gb = {"bxr": bxr, "byr": byr, "bxz": bxz, "byz": byz,
          "bxg": bxg, "byg": byg}
    in_maps = [prep_inputs_core(c, key, pe, key_index, key_padding_mask,
                                in_proj_w, in_proj_b, out_w, out_b, gw, gb)
               for c in range(8)]
    res = run_bass_kernel_spmd(nc, in_maps, core_ids=list(range(8)))
    out, attn_w = postprocess(res.results)
    return out.astype(np.float32), attn_w.astype(np.float32)

